# revision 3
# baseline (speedup 1.0000x reference)
"""Trainium2 Bass kernel for sparse multi-head edge attention.

Computation (per the nn.Module):
    Q = Fa @ Wq.T, K = Fb @ Wk.T, V = Fb @ Wv.T   (H=8 heads x 32)
    per edge e: logit[e,h] = <Q[a_e,h,:], K[b_e,h,:]> / sqrt(32)
    segmented softmax over edges per query; out = Fa + (weighted V) @ Wproj.T
    Softmax runs without max-subtraction (|logit| <= ~6, fp32-exp safe);
    queries are sharded across the 8 cores so it is fully core-local.

Key design points:
  - Host bin-packs queries into 392 (core, block) bins so every block has
    <= 1408 lo-edges and <= 768 hi-edges (128-padded streams, zero
    inter-core imbalance).  Slots/core: 106624 (833 tiles) vs 119k.
  - Gathers use all 4 SWDGE queues (desc-gen runs on distinct Q7 cpu
    pairs concurrently, ~4x the serial descriptor rate).
  - Fused table row = K fp16 | V fp16 (1KB), gathered once per edge
    across all 4 SWDGE queues.
  - sel / selT one-hot matrices are precomputed on host (fp8) and
    streamed from DRAM; no per-tile is_equal, no ARELT stream, no exp
    bias (pad slots have all-zero sel columns so they contribute 0).
  - Vector ops run on 4..5-tile groups; exp on 4..5 tiles in one
    activation.
"""

import math

import numpy as np
import ml_dtypes

P = 128
H = 8
DH = 32
CDIM = 256
NA = 50000
NB = 50000
NCORES = 8
NBLK = 49
NBINS = NCORES * NBLK
NQP = NBLK * P                 # 6272 padded queries per core
SPLIT = 32768
KV_ROWS = ((NB + P - 1) // P) * P   # 50048
KVHI_ROWS = KV_ROWS - SPLIT         # 17280
LO_CAP = 1408                  # 11 tiles
HI_CAP = 768                   # 6 tiles
CAPB = LO_CAP + HI_CAP         # 2176 slots / block (17 tiles)
NTILE = CAPB // P              # 17
SLOTS = NBLK * CAPB            # 106624 per core
ROWE = 512                     # table row elems (fp16): K 256 | V 256
CHUNK = 2048
SCALE = 1.0 / math.sqrt(DH)
GROUPS = [(0, 4), (4, 4), (8, 4), (12, 5)]   # 17 tiles per block

F16 = np.float16
F32 = np.float32
F8 = ml_dtypes.float8_e4m3


def _pack(a_idx, b_idx):
    """Assign each query to a (core, block) bin; balance lo/hi edge counts."""
    lo_mask = b_idx < SPLIT
    lo_deg = np.bincount(a_idx[lo_mask], minlength=NA)
    hi_deg = np.bincount(a_idx[~lo_mask], minlength=NA)
    order = np.argsort(-(lo_deg + hi_deg), kind="stable")

    bin_lo = np.zeros(NBINS)
    bin_hi = np.zeros(NBINS)
    bin_nq = np.zeros(NBINS, np.int64)
    assign = np.empty(NA, np.int64)
    for q in order:
        dl, dh = lo_deg[q], hi_deg[q]
        load = bin_lo / LO_CAP + bin_hi / HI_CAP
        feas = (bin_nq < P) & (bin_lo + dl <= LO_CAP) & (bin_hi + dh <= HI_CAP)
        if not feas.any():
            raise RuntimeError(f"bin-packing infeasible for query {q}")
        load[~feas] = np.inf
        m = int(np.argmin(load))
        assign[q] = m
        bin_lo[m] += dl
        bin_hi[m] += dh
        bin_nq[m] += 1
    assert bin_lo.max() <= LO_CAP and bin_hi.max() <= HI_CAP
    return assign


def wrap16(arr):
    w = arr.reshape(-1, 16).T
    return np.tile(w, (8, 1)).copy()


def preprocess(Fa, Fb, a_idx, b_idx, Wq, Wk, Wv, Wproj):
    a_idx = np.asarray(a_idx).astype(np.int64)
    b_idx = np.asarray(b_idx).astype(np.int64)
    Fa = np.asarray(Fa, F32)
    Fb = np.asarray(Fb, F32)

    assign = _pack(a_idx, b_idx)          # [NA] -> bin 0..391
    core_q = assign // NBLK
    blk_q = assign % NBLK

    # rank of each query within its bin (by query id)
    qorder = np.argsort(assign, kind="stable")
    bin_sorted = assign[qorder]
    starts = np.searchsorted(bin_sorted, np.arange(NBINS))
    q_rel = np.empty(NA, np.int64)
    q_rel[qorder] = np.arange(NA) - starts[bin_sorted]

    # qperm[m, j*128+r] = original query id (or -1)
    qperm = np.full((NCORES, NQP), -1, np.int64)
    qperm[core_q, blk_q * P + q_rel] = np.arange(NA)

    # edge slots
    m_e = core_q[a_idx]
    j_e = blk_q[a_idx]
    qr_e = q_rel[a_idx]
    hi = b_idx >= SPLIT
    gid = (m_e * NBLK + j_e) * 2 + hi
    eorder = np.lexsort((b_idx, gid))
    counts = np.bincount(gid, minlength=NBINS * 2)
    gstart = np.concatenate([[0], np.cumsum(counts)])[:-1]
    rank = np.empty(len(a_idx), np.int64)
    rank[eorder] = np.arange(len(a_idx)) - gstart[gid[eorder]]

    slot = np.where(hi, j_e * CAPB + LO_CAP + rank, j_e * CAPB + rank)

    arel = np.full((NCORES, SLOTS), 255, np.int64)
    arel[m_e, slot] = qr_e

    idxlo = np.zeros((NCORES, NBLK * LO_CAP), np.int16)
    idxhi = np.zeros((NCORES, NBLK * HI_CAP), np.int16)
    lo_m = ~hi
    idxlo[m_e[lo_m], j_e[lo_m] * LO_CAP + rank[lo_m]] = b_idx[lo_m].astype(np.int16)
    idxhi[m_e[hi], j_e[hi] * HI_CAP + rank[hi]] = (b_idx[hi] - SPLIT).astype(np.int16)

    shared = {
        "FbT": np.zeros((CDIM, KV_ROWS), F16),
        "WqT": Wq.T.astype(F16).copy(),
        "WKVT": np.concatenate([Wk.T, Wv.T], axis=1).astype(F16).copy(),
        "WprojT": Wproj.T.astype(F16).copy(),
        "IDENT": np.eye(P, dtype=F16),
    }
    shared["FbT"][:, :NB] = Fb.T.astype(F16)

    qid128 = np.arange(P)
    per_core = []
    for m in range(NCORES):
        ar = arel[m]
        # SELT[q, s] = (arel[s] == q)
        selt = (ar[None, :] == qid128[:, None]).astype(F8)
        # SEL[e, t*128+q] = (arel[t*128+e] == q)
        ar_t = ar.reshape(-1, P)
        sel = (ar_t[:, :, None] == qid128[None, None, :]).astype(F8)
        sel = np.ascontiguousarray(sel.transpose(1, 0, 2)).reshape(P, -1)

        qp = qperm[m]
        valid = qp >= 0
        FaT = np.zeros((CDIM, NQP), F16)
        FaT[:, valid] = Fa[qp[valid]].T.astype(F16)
        FaRes = np.zeros((NQP, CDIM), F16)
        FaRes[valid] = Fa[qp[valid]].astype(F16)
        per_core.append({
            "FaT": FaT,
            "FaRes": FaRes,
            "SEL": sel,
            "SELT": selt,
            "IDXLO": wrap16(idxlo[m]),
            "IDXHI": wrap16(idxhi[m]),
        })

    meta = {"qperm": qperm}
    return meta, shared, per_core


def build_program():
    import concourse.bacc as bacc
    import concourse.mybir as mybir
    from concourse.tile import TileContext
    from concourse import library_config

    dt = mybir.dt
    AluOp = mybir.AluOpType
    nc = bacc.Bacc("TRN2", target_bir_lowering=False, debug=False,
                   num_devices=NCORES, num_swdge_queues=4)

    FbT_t = nc.dram_tensor("FbT", [CDIM, KV_ROWS], dt.float16, kind="ExternalInput")
    FaT_t = nc.dram_tensor("FaT", [CDIM, NQP], dt.float16, kind="ExternalInput")
    FaRes_t = nc.dram_tensor("FaRes", [NQP, CDIM], dt.float16, kind="ExternalInput")
    WqT_t = nc.dram_tensor("WqT", [CDIM, CDIM], dt.float16, kind="ExternalInput")
    WKVT_t = nc.dram_tensor("WKVT", [CDIM, 2 * CDIM], dt.float16, kind="ExternalInput")
    WprojT_t = nc.dram_tensor("WprojT", [CDIM, CDIM], dt.float16, kind="ExternalInput")
    IDENT_t = nc.dram_tensor("IDENT", [P, P], dt.float16, kind="ExternalInput")
    SEL_t = nc.dram_tensor("SEL", [P, SLOTS], dt.float8e4, kind="ExternalInput")
    SELT_t = nc.dram_tensor("SELT", [P, SLOTS], dt.float8e4, kind="ExternalInput")
    IDXLO_t = nc.dram_tensor("IDXLO", [P, NBLK * LO_CAP // 16], dt.int16,
                             kind="ExternalInput")
    IDXHI_t = nc.dram_tensor("IDXHI", [P, NBLK * HI_CAP // 16], dt.int16,
                             kind="ExternalInput")

    KVlo = nc.dram_tensor("KVlo", [SPLIT, ROWE], dt.float16, kind="Internal")
    KVhi = nc.dram_tensor("KVhi", [KVHI_ROWS, ROWE], dt.float16, kind="Internal")
    OUT_t = nc.dram_tensor("OUT", [NQP, CDIM], dt.float16, kind="ExternalOutput")

    with TileContext(nc) as tc:
        nc.gpsimd.load_library(library_config.mlp)
        with tc.tile_pool(name="res", bufs=1) as rpool:
            wq = rpool.tile([P, 2, CDIM], dt.float16, tag="wq")
            wkv = rpool.tile([P, 2, 2 * CDIM], dt.float16, tag="wkv")
            wproj = rpool.tile([P, 2, CDIM], dt.float16, tag="wproj")
            ident = rpool.tile([P, P], dt.float16, tag="ident")
            nc.sync.dma_start(out=wq[:, 0, :], in_=WqT_t[0:P, :])
            nc.sync.dma_start(out=wq[:, 1, :], in_=WqT_t[P:2 * P, :])
            nc.sync.dma_start(out=wkv[:, 0, :], in_=WKVT_t[0:P, :])
            nc.sync.dma_start(out=wkv[:, 1, :], in_=WKVT_t[P:2 * P, :])
            nc.sync.dma_start(out=wproj[:, 0, :], in_=WprojT_t[0:P, :])
            nc.sync.dma_start(out=wproj[:, 1, :], in_=WprojT_t[P:2 * P, :])
            nc.sync.dma_start(out=ident[:], in_=IDENT_t[:, :])
            idxlo = rpool.tile([P, NBLK * LO_CAP // 16], dt.int16, tag="idxlo")
            idxhi = rpool.tile([P, NBLK * HI_CAP // 16], dt.int16, tag="idxhi")
            nc.sync.dma_start(out=idxlo[:], in_=IDXLO_t[:, :])
            nc.sync.dma_start(out=idxhi[:], in_=IDXHI_t[:, :])
            qres = rpool.tile([P, NBLK, CDIM], dt.float16, tag="qres")

            # ---- Phase A: Q (SBUF) + fused KV table (DRAM) ----
            with tc.tile_pool(name="bld", bufs=2) as bpool, \
                 tc.tile_pool(name="psA", bufs=4, space="PSUM") as psA:
                for c0 in range(0, NQP, CHUNK):
                    nsub = min(CHUNK, NQP - c0) // P
                    ft = bpool.tile([P, 2, CHUNK], dt.float16, tag="ft")
                    nc.sync.dma_start(out=ft[:, 0, :nsub * P],
                                      in_=FaT_t[0:P, c0:c0 + nsub * P])
                    nc.sync.dma_start(out=ft[:, 1, :nsub * P],
                                      in_=FaT_t[P:2 * P, c0:c0 + nsub * P])
                    for s in range(nsub):
                        ps = psA.tile([P, 2 * CDIM], dt.float32, tag="psA")
                        nc.tensor.matmul(ps[:, 0:CDIM], ft[:, 0, s * P:(s + 1) * P],
                                         wq[:, 0, :], start=True, stop=False)
                        nc.tensor.matmul(ps[:, 0:CDIM], ft[:, 1, s * P:(s + 1) * P],
                                         wq[:, 1, :], start=False, stop=True)
                        nc.scalar.copy(out=qres[:, c0 // P + s, :], in_=ps[:, 0:CDIM])
                for c0 in range(0, KV_ROWS, CHUNK):
                    nsub = min(CHUNK, KV_ROWS - c0) // P
                    ft = bpool.tile([P, 2, CHUNK], dt.float16, tag="ft")
                    nc.sync.dma_start(out=ft[:, 0, :nsub * P],
                                      in_=FbT_t[0:P, c0:c0 + nsub * P])
                    nc.sync.dma_start(out=ft[:, 1, :nsub * P],
                                      in_=FbT_t[P:2 * P, c0:c0 + nsub * P])
                    ob = bpool.tile([P, CHUNK // P, ROWE], dt.float16, tag="ob")
                    for s in range(nsub):
                        ps = psA.tile([P, 2 * CDIM], dt.float32, tag="psA")
                        nc.tensor.matmul(ps[:], ft[:, 0, s * P:(s + 1) * P],
                                         wkv[:, 0, :], start=True, stop=False)
                        nc.tensor.matmul(ps[:], ft[:, 1, s * P:(s + 1) * P],
                                         wkv[:, 1, :], start=False, stop=True)
                        nc.scalar.copy(out=ob[:, s, 0:CDIM], in_=ps[:, 0:CDIM])
                        nc.vector.tensor_scalar_mul(out=ob[:, s, CDIM:2 * CDIM],
                                                    in0=ps[:, CDIM:2 * CDIM],
                                                    scalar1=1.0)
                    if c0 < SPLIT:
                        dst = KVlo[c0:c0 + nsub * P, :]
                    else:
                        dst = KVhi[c0 - SPLIT:c0 - SPLIT + nsub * P, :]
                    nc.sync.dma_start(
                        out=dst.rearrange("(s p) d -> p s d", p=P),
                        in_=ob[:, :nsub, :])

            # ---- Phase B ----
            with tc.tile_pool(name="gat", bufs=4) as gpool, \
                 tc.tile_pool(name="sp", bufs=3) as spool, \
                 tc.tile_pool(name="wrk", bufs=2) as wpool, \
                 tc.tile_pool(name="fin", bufs=2) as fpool, \
                 tc.tile_pool(name="psB", bufs=1, space="PSUM") as psB, \
                 tc.tile_pool(name="psD", bufs=2, space="PSUM") as psD:
                for j in range(NBLK):
                    kve = gpool.tile([P, NTILE, ROWE], dt.float16, tag="kve")
                    nc.gpsimd.dma_gather(
                        out_ap=kve[:, 0:LO_CAP // P, :], in_ap=KVlo[:, :],
                        idxs_ap=idxlo[:, j * (LO_CAP // 16):(j + 1) * (LO_CAP // 16)],
                        num_idxs=LO_CAP, num_idxs_reg=LO_CAP,
                        elem_size=ROWE, single_packet=False,
                        queue_num=(2 * j) % 4)
                    nc.gpsimd.dma_gather(
                        out_ap=kve[:, LO_CAP // P:NTILE, :], in_ap=KVhi[:, :],
                        idxs_ap=idxhi[:, j * (HI_CAP // 16):(j + 1) * (HI_CAP // 16)],
                        num_idxs=HI_CAP, num_idxs_reg=HI_CAP,
                        elem_size=ROWE, single_packet=False,
                        queue_num=(2 * j + 1) % 4)
                    selb = spool.tile([P, CAPB], dt.float8e4, tag="selb")
                    seltb = spool.tile([P, CAPB], dt.float8e4, tag="seltb")
                    nc.sync.dma_start(out=selb[:],
                                      in_=SEL_t[:, j * CAPB:(j + 1) * CAPB])
                    nc.sync.dma_start(out=seltb[:],
                                      in_=SELT_t[:, j * CAPB:(j + 1) * CAPB])

                    dnps = psD.tile([P, 264], dt.float32, tag="dn")
                    for (t0, G) in GROUPS:
                        prodg = wpool.tile([P, 5, CDIM], dt.float16, tag="prodg")
                        npair = (G + 1) // 2
                        for pi in range(npair):
                            n = min(2, G - pi * 2)
                            bank = psB.tile([P, 512], dt.float32,
                                            tag=f"qe{'ABC'[pi]}")
                            for k in range(n):
                                t = t0 + pi * 2 + k
                                nc.tensor.matmul(
                                    bank[:, k * CDIM:(k + 1) * CDIM],
                                    seltb[:, t * P:(t + 1) * P],
                                    qres[:, j, :], start=True, stop=True)
                            qe_sb = wpool.tile([P, 2, CDIM], dt.float16,
                                               tag="qe_sb")
                            nc.scalar.copy(out=qe_sb[:, 0:n, :],
                                           in_=bank[:, 0:n * CDIM].rearrange(
                                               "p (t d) -> p t d", d=CDIM))
                            nc.vector.tensor_tensor(
                                out=prodg[:, pi * 2:pi * 2 + n, :],
                                in0=qe_sb[:, 0:n, :],
                                in1=kve[:, t0 + pi * 2:t0 + pi * 2 + n, 0:CDIM],
                                op=AluOp.mult)
                        logitsg = wpool.tile([P, 5, H], dt.float32, tag="logitsg")
                        nc.vector.tensor_reduce(
                            out=logitsg[:, 0:G, :],
                            in_=prodg[:, 0:G, :].rearrange(
                                "p t (h d) -> p t h d", d=DH),
                            axis=mybir.AxisListType.X, op=AluOp.add)
                        exwvg = wpool.tile([P, 5, H + CDIM], dt.float16, tag="exwvg")
                        nc.scalar.activation(
                            out=exwvg[:, 0:G, 0:H], in_=logitsg[:, 0:G, :],
                            func=mybir.ActivationFunctionType.Exp, scale=SCALE)
                        exq = wpool.tile([P, 5, H, DH], dt.float16, tag="exq")
                        nc.scalar.activation(
                            out=exq[:, 0:G, :, :],
                            in_=logitsg[:, 0:G, :].unsqueeze(3)
                                .to_broadcast([P, G, H, DH]),
                            func=mybir.ActivationFunctionType.Exp, scale=SCALE)
                        nc.vector.tensor_tensor(
                            out=exwvg[:, 0:G, H:H + CDIM],
                            in0=kve[:, t0:t0 + G, CDIM:2 * CDIM],
                            in1=exq[:, 0:G, :, :].rearrange(
                                "p t h d -> p t (h d)"),
                            op=AluOp.mult)
                        for k in range(G):
                            t = t0 + k
                            nc.tensor.matmul(dnps[:], selb[:, t * P:(t + 1) * P],
                                             exwvg[:, k, :],
                                             start=(t == 0), stop=(t == NTILE - 1))

                    den = fpool.tile([P, H], dt.float32, tag="den")
                    nc.vector.tensor_scalar_max(out=den[:], in0=dnps[:, 0:H],
                                                scalar1=1e-30)
                    rec = fpool.tile([P, H], dt.float32, tag="rec")
                    nc.vector.reciprocal(out=rec[:], in_=den[:])
                    s_sb = fpool.tile([P, CDIM], dt.float16, tag="s_sb")
                    nc.vector.tensor_tensor(
                        out=s_sb[:], in0=dnps[:, H:H + CDIM],
                        in1=rec[:].unsqueeze(2).to_broadcast([P, H, DH]),
                        op=AluOp.mult)
                    st_ps = psB.tile([P, 2, P], dt.float16, tag="stps")
                    nc.tensor.transpose(st_ps[:, 0, :], s_sb[:, 0:P], ident[:])
                    nc.tensor.transpose(st_ps[:, 1, :], s_sb[:, P:2 * P], ident[:])
                    st_sb = fpool.tile([P, 2, P], dt.float16, tag="st_sb")
                    nc.scalar.copy(out=st_sb[:], in_=st_ps[:])
                    outps = psB.tile([P, 512], dt.float32, tag="qeC")
                    nc.tensor.matmul(outps[:, 0:CDIM], st_sb[:, 0, :], wproj[:, 0, :],
                                     start=True, stop=False)
                    nc.tensor.matmul(outps[:, 0:CDIM], st_sb[:, 1, :], wproj[:, 1, :],
                                     start=False, stop=True)
                    fa_t = fpool.tile([P, CDIM], dt.float16, tag="fa_t")
                    nc.sync.dma_start(out=fa_t[:], in_=FaRes_t[j * P:(j + 1) * P, :])
                    res = fpool.tile([P, CDIM], dt.float16, tag="res")
                    nc.vector.tensor_tensor(out=res[:], in0=outps[:, 0:CDIM],
                                            in1=fa_t[:], op=AluOp.add)
                    nc.sync.dma_start(out=OUT_t[j * P:(j + 1) * P, :], in_=res[:])

    nc.compile()
    return nc


TRACE = False
LAST_RESULT = None
_PROGRAM = None


def kernel(**inputs):
    global LAST_RESULT, _PROGRAM
    from concourse.bass_utils import run_bass_kernel_spmd

    meta, shared, per_core = preprocess(**inputs)
    if _PROGRAM is None:
        _PROGRAM = build_program()
    in_maps = [dict(shared, **pc) for pc in per_core]
    res = run_bass_kernel_spmd(_PROGRAM, in_maps, core_ids=list(range(NCORES)),
                               trace=TRACE)
    LAST_RESULT = res
    out = np.empty((NA, CDIM), F32)
    qperm = meta["qperm"]
    for m in range(NCORES):
        qp = qperm[m]
        valid = qp >= 0
        out[qp[valid]] = res.results[m]["OUT"][valid].astype(F32)
    return out
